# revision 11
# baseline (speedup 1.0000x reference)
"""BCRNN layer (bidirectional convolutional RNN) on 8 Trainium2 NeuronCores.

Problem: nb=1, nc=2, nt=12, nx=160, ny=160, hid=64, K=3, reflect padding,
complex conv decomposed into real convs, modReLU activation, forward +
backward temporal scans, output = sum of the two scans stacked (r, i).

Sharding: spatial rows (nx) split 8 ways (20 rows/core).  The temporal scan
runs locally per core; the 1-row conv halo of the recurrent state is
exchanged each step with an AllGather of boundary rows + per-core indirect
gather (per-core routing lives in an index input tensor so the single SPMD
program stays uniform; global-edge reflect rows are sourced the same way).

Compute strategy per conv: 9 shifted fp16 matmuls (K=128 = stacked
real/imag channels, M=128 = stacked real/imag outputs) accumulated in PSUM.
All matmul rhs operands are single CONTIGUOUS runs over the padded-row
layout (row stride == row length 162), with psum rows also 162 wide; the
two junk edge columns per row are overwritten by the reflect fixup.
i2h conv is one K=36 matmul over a host-built im2col of the 2-channel
input.  modReLU magnitude pairing (zr^2+zi^2 across partition halves) is
one extra 0/1-matrix matmul; z^2 is computed scaled by 2^-12 to stay in
fp16 range and unscaled exactly inside the Sqrt activation.
"""
import os

import numpy as np

import concourse.bass as bass
import concourse.bacc as bacc
import concourse.tile as tile
import concourse.mybir as mybir
from concourse.bass_utils import run_bass_kernel_spmd

P = 128
NC_CORES = 8
NT = 12
NX, NY = 160, 160
HID = 64
ROWS = NX // NC_CORES      # 20 owned rows per core
TR = ROWS + 2              # 22 tile rows (with halo)
YP = NY + 2                # 162 padded cols
FS = TR * YP + 2           # flat h/iter tile size (+1 guard col each side)
NSLOT = 8                  # cc slots: 4 fwd + 4 bwd boundary rows

f32 = mybir.dt.float32
f16 = mybir.dt.float16
i32 = mybir.dt.int32
AF = mybir.ActivationFunctionType
ALU = mybir.AluOpType

# conv groups (r0, nrows) over owned rows; scan order: boundary groups first
SCAN_GROUPS = [(0, 3), (17, 3), (3, 3), (6, 3), (9, 3), (12, 3), (15, 2)]
PRE_GROUPS = [(0, 3), (3, 3), (6, 3), (9, 3), (12, 3), (15, 3), (18, 2)]
CC_ROWS = [1, 2, 19, 20]   # contributed tile rows per direction
GN = 3 * YP                # max psum group width (486)

_CACHED = {}
TRACE = False
LAST = {}


def _row(q):
    """flat offset of (tile row q, padded col 0) in an FS-sized tile"""
    return 1 + q * YP


def _build():
    STAGE = int(os.environ.get("STAGE", "4"))
    if "nc" in _CACHED:
        return _CACHED["nc"]
    nc = bacc.Bacc(None, target_bir_lowering=False, debug=False,
                   num_devices=NC_CORES)

    # ---- I/O ----
    iter_il = nc.dram_tensor("iter_il", [NT, P, TR * YP], f16, kind="ExternalInput")
    im2col = nc.dram_tensor("im2col", [NT, 36, ROWS * YP], f16, kind="ExternalInput")
    wh2h = nc.dram_tensor("wh2h", [9, P, P], f16, kind="ExternalInput")
    wih = nc.dram_tensor("wih", [9, P, P], f16, kind="ExternalInput")
    wi2h = nc.dram_tensor("wi2h", [36, P], f16, kind="ExternalInput")
    pmat = nc.dram_tensor("pmat", [P, P], f16, kind="ExternalInput")
    b_pre = nc.dram_tensor("b_pre", [P, 1], f32, kind="ExternalInput")
    b_mod = nc.dram_tensor("b_mod", [P, 1], f32, kind="ExternalInput")
    hidx = nc.dram_tensor("hidx", [P, 4], i32, kind="ExternalInput")
    out = nc.dram_tensor("out", [NT, P, ROWS, NY], f32, kind="ExternalOutput")

    # ---- internal DRAM ----
    predr = nc.dram_tensor("predr", [NT, P, ROWS * YP], f32)
    sav = nc.dram_tensor("sav", [NT, P, ROWS, NY], f16)

    with tile.TileContext(nc) as tc:
        with (
            tc.tile_pool(name="wp", bufs=1) as wp,
            tc.tile_pool(name="dram", bufs=2, space="DRAM") as dram,
            tc.tile_pool(name="cps", bufs=4, space="PSUM") as cps,
            tc.tile_pool(name="mps", bufs=2, space="PSUM") as mps,
        ):
            # weights / constants
            wh = [wp.tile([P, P], f16, tag=f"wh{k}", name=f"wh{k}") for k in range(9)]
            wi = [wp.tile([P, P], f16, tag=f"wi{k}", name=f"wi{k}") for k in range(9)]
            for k in range(9):
                nc.sync.dma_start(out=wh[k][:], in_=wh2h[k])
                nc.sync.dma_start(out=wi[k][:], in_=wih[k])
            w36 = wp.tile([36, P], f16, tag="w36")
            nc.sync.dma_start(out=w36[:], in_=wi2h[:])
            pm = wp.tile([P, P], f16, tag="pm")
            nc.sync.dma_start(out=pm[:], in_=pmat[:])
            bp = wp.tile([P, 1], f32, tag="bp")
            nc.sync.dma_start(out=bp[:], in_=b_pre[:])
            bm = wp.tile([P, 1], f32, tag="bm")
            nc.sync.dma_start(out=bm[:], in_=b_mod[:])
            hix = wp.tile([P, 4], i32, tag="hix")
            nc.sync.dma_start(out=hix[:], in_=hidx[:])
            epst = wp.tile([P, 1], f32, tag="epst")
            nc.vector.memset(epst[:], 1e-6)

            # ---------------- phase 1: pre[t] = ih(iter) + i2h(input) + bias
            with (
                tc.tile_pool(name="itp", bufs=2) as itp,
                tc.tile_pool(name="icp", bufs=2) as icp,
                tc.tile_pool(name="prs", bufs=2) as prs,
            ):
                for t in range(NT):
                    it = itp.tile([P, FS], f16, tag="it")
                    nc.sync.dma_start(out=it[:, 1:1 + TR * YP], in_=iter_il[t])
                    ic = icp.tile([36, ROWS * YP], f16, tag="ic")
                    nc.sync.dma_start(out=ic[:], in_=im2col[t])
                    stage = prs.tile([P, ROWS * YP], f32, tag="stage")
                    for r0, nr in PRE_GROUPS:
                        n = nr * YP
                        ps = cps.tile([P, GN], f32, tag="cv")
                        for tap in range(9):
                            dxi, dyi = divmod(tap, 3)
                            base = (r0 + dxi) * YP + dyi
                            nc.tensor.matmul(
                                out=ps[:, :n],
                                lhsT=wi[tap][:],
                                rhs=it[:, base:base + n],
                                start=(tap == 0), stop=False,
                            )
                        nc.tensor.matmul(
                            out=ps[:, :n],
                            lhsT=w36[:],
                            rhs=ic[:, r0 * YP:r0 * YP + n],
                            start=False, stop=True,
                        )
                        nc.scalar.activation(stage[:, r0 * YP:r0 * YP + n],
                                             ps[:, :n],
                                             AF.Identity, bias=bp[:, 0:1], scale=1.0)
                    nc.sync.dma_start(out=predr[t], in_=stage[:])

            # ---------------- phase 2: bidirectional scan
            with (
                tc.tile_pool(name="hfp", bufs=2) as hfp,
                tc.tile_pool(name="hbp", bufs=2) as hbp,
                tc.tile_pool(name="pin", bufs=3) as pinp,
                tc.tile_pool(name="zp", bufs=2) as zp,
                tc.tile_pool(name="z2p", bufs=2) as z2p,
                tc.tile_pool(name="mgp", bufs=2) as mgp,
                tc.tile_pool(name="nmp", bufs=2) as nmp,
                tc.tile_pool(name="rcp", bufs=2) as rcp,
                tc.tile_pool(name="scp", bufs=2) as scp,
                tc.tile_pool(name="svp", bufs=2) as svp,
                tc.tile_pool(name="obp", bufs=2) as obp,
            ):
                hf_prev = hb_prev = None
                for s in range(NT if STAGE > 1 else 0):
                    new_state = {}
                    for dire, (hpool, h_prev) in (("f", (hfp, hf_prev)),
                                                  ("b", (hbp, hb_prev))):
                        t_d = s if dire == "f" else NT - 1 - s
                        pin = pinp.tile([P, ROWS * YP], f32, tag="pin")
                        nc.sync.dma_start(out=pin[:], in_=predr[t_d])
                        h_next = hpool.tile([P, FS], f16, tag=f"h{dire}")
                        if STAGE == 2:
                            nc.vector.memset(h_next[:, _row(0):_row(1)], 0.0)
                            nc.vector.memset(h_next[:, _row(TR - 1):_row(TR)], 0.0)
                        for r0, nr in SCAN_GROUPS:
                            n = nr * YP
                            if s > 0:
                                ps = cps.tile([P, GN], f32, tag="cv")
                                for tap in range(9):
                                    dxi, dyi = divmod(tap, 3)
                                    base = (r0 + dxi) * YP + dyi
                                    nc.tensor.matmul(
                                        out=ps[:, :n],
                                        lhsT=wh[tap][:],
                                        rhs=h_prev[:, base:base + n],
                                        start=(tap == 0), stop=(tap == 8),
                                    )
                                z = zp.tile([P, GN], f32, tag="z")
                                nc.vector.tensor_tensor(
                                    out=z[:, :n], in0=ps[:, :n],
                                    in1=pin[:, r0 * YP:r0 * YP + n], op=ALU.add)
                                zv = z[:, :n]
                            else:
                                zv = pin[:, r0 * YP:r0 * YP + n]
                            z2 = z2p.tile([P, GN], f16, tag="z2")
                            nc.scalar.activation(z2[:, :n], zv, AF.Square,
                                                 scale=0.015625)
                            mp = mps.tile([P, GN], f32, tag="mg")
                            nc.tensor.matmul(out=mp[:, :n], lhsT=pm[:],
                                             rhs=z2[:, :n],
                                             start=True, stop=True)
                            mag = mgp.tile([P, GN], f32, tag="mag")
                            nc.scalar.activation(mag[:, :n], mp[:, :n],
                                                 AF.Sqrt, bias=epst[:, 0:1],
                                                 scale=4096.0)
                            num = nmp.tile([P, GN], f32, tag="num")
                            nc.scalar.activation(num[:, :n], mag[:, :n],
                                                 AF.Relu, bias=bm[:, 0:1])
                            rec = rcp.tile([P, GN], f32, tag="rec")
                            nc.vector.reciprocal_approx_fast(
                                out=rec[:, :n], in_=mag[:, :n])
                            sc = scp.tile([P, GN], f32, tag="sc")
                            nc.vector.tensor_tensor(out=sc[:, :n],
                                                    in0=num[:, :n],
                                                    in1=rec[:, :n],
                                                    op=ALU.mult)
                            hb_ = _row(r0 + 1)
                            nc.vector.tensor_tensor(
                                out=h_next[:, hb_:hb_ + n],
                                in0=zv, in1=sc[:, :n], op=ALU.mult)
                        if s < NT - 1:
                            # y-edge reflect on owned rows (also fixes junk
                            # psum edge columns)
                            vr = h_next[:, _row(1):_row(TR - 1)].rearrange(
                                "p (r y) -> p r y", y=YP)
                            nc.vector.tensor_copy(out=vr[:, :, 0:1],
                                                  in_=vr[:, :, 2:3])
                            nc.vector.tensor_copy(out=vr[:, :, YP - 1:YP],
                                                  in_=vr[:, :, YP - 3:YP - 2])
                        new_state[dire] = h_next

                    hf_next, hb_next = new_state["f"], new_state["b"]

                    if s < NT - 1 and STAGE >= 3:
                        # halo exchange
                        cc_in = dram.tile([NSLOT * P, YP], f16, tag="cci")
                        cc_out = dram.tile([NC_CORES * NSLOT * P, YP], f16,
                                           addr_space="Shared", tag="cco")
                        cci_v = cc_in[:].rearrange("(s p) y -> s p y", p=P)
                        for di, h_n in ((0, hf_next), (1, hb_next)):
                            for si, row in enumerate(CC_ROWS):
                                nc.sync.dma_start(
                                    out=cci_v[di * 4 + si],
                                    in_=h_n[:, _row(row):_row(row + 1)])
                        nc.gpsimd.collective_compute(
                            "AllGather", ALU.bypass,
                            replica_groups=[list(range(NC_CORES))],
                            ins=[cc_in[:].opt()], outs=[cc_out[:].opt()],
                        )
                        for k, (h_n, row) in enumerate(
                                ((hf_next, 0), (hf_next, TR - 1),
                                 (hb_next, 0), (hb_next, TR - 1))):
                            nc.gpsimd.indirect_dma_start(
                                out=h_n[:, _row(row):_row(row + 1)],
                                out_offset=None,
                                in_=cc_out[:],
                                in_offset=bass.IndirectOffsetOnAxis(
                                    ap=hix[:, k:k + 1], axis=0),
                            )

                    # save / combine
                    def _ivw(h_n):
                        return h_n[:, _row(1):_row(TR - 1)].rearrange(
                            "p (r y) -> p r y", y=YP)[:, :, 1:1 + NY]

                    if s <= 5:
                        nc.sync.dma_start(out=sav[s], in_=_ivw(hf_next))
                        nc.sync.dma_start(out=sav[NT - 1 - s], in_=_ivw(hb_next))
                    else:
                        for h_n, t_o in ((hf_next, s), (hb_next, NT - 1 - s)):
                            ld = svp.tile([P, ROWS, NY], f16, tag="ld")
                            nc.sync.dma_start(out=ld[:], in_=sav[t_o])
                            ob = obp.tile([P, ROWS, NY], f32, tag="ob")
                            nc.vector.tensor_tensor(
                                out=ob[:], in0=_ivw(h_n),
                                in1=ld[:], op=ALU.add)
                            nc.sync.dma_start(out=out[t_o], in_=ob[:])

                    hf_prev, hb_prev = hf_next, hb_next

    nc.compile()
    _CACHED["nc"] = nc
    return nc


def _complex_lhsT(wr, wi_):
    """[O, I, 3, 3] complex pair -> per-tap lhsT [9, 2*I, 2*O]."""
    O, I = wr.shape[:2]
    lhsT = np.zeros((9, 2 * I, 2 * O), np.float32)
    for tap in range(9):
        kx, ky = divmod(tap, 3)
        lhsT[tap, :I, :O] = wr[:, :, kx, ky].T
        lhsT[tap, I:, :O] = -wi_[:, :, kx, ky].T
        lhsT[tap, :I, O:] = wi_[:, :, kx, ky].T
        lhsT[tap, I:, O:] = wr[:, :, kx, ky].T
    return lhsT


def kernel(**inputs):
    inp_r = np.asarray(inputs["input_r"], np.float32)
    inp_i = np.asarray(inputs["input_i"], np.float32)
    itr_r = np.asarray(inputs["iter_r"], np.float32)
    itr_i = np.asarray(inputs["iter_i"], np.float32)

    # ---- weights ----
    wh2h = _complex_lhsT(np.asarray(inputs["w_h2h_r"]), np.asarray(inputs["w_h2h_i"]))
    wih = _complex_lhsT(np.asarray(inputs["w_ih_r"]), np.asarray(inputs["w_ih_i"]))
    w4 = _complex_lhsT(np.asarray(inputs["w_i2h_r"]), np.asarray(inputs["w_i2h_i"]))
    wi2h = np.ascontiguousarray(w4.reshape(36, P))
    pmat = np.zeros((P, P), np.float32)
    for k in range(P):
        pmat[k, k % HID] = 1.0
        pmat[k, HID + k % HID] = 1.0
    b_pre = np.concatenate([
        inputs["b_i2h_r"] + inputs["b_ih_r"] + inputs["b_h2h_r"],
        inputs["b_i2h_i"] + inputs["b_ih_i"] + inputs["b_h2h_i"],
    ]).astype(np.float32)[:, None]
    b_mod = np.tile(np.asarray(inputs["mod_b"], np.float32), 2)[:, None]

    # ---- activations, reflect-padded [t, ch, xpad, ypad] ----
    itg = np.concatenate([itr_r[0], itr_i[0]], axis=0).transpose(1, 0, 2, 3)
    itg = np.pad(itg, ((0, 0), (0, 0), (1, 1), (1, 1)), mode="reflect")
    ing = np.concatenate([inp_r[0], inp_i[0]], axis=0).transpose(1, 0, 2, 3)
    ing = np.pad(ing, ((0, 0), (0, 0), (1, 1), (1, 1)), mode="reflect")
    # one extra (junk) col each side for the 162-wide im2col windows
    ing2 = np.pad(ing, ((0, 0), (0, 0), (0, 0), (1, 1)), mode="edge")

    in_maps = []
    for c in range(NC_CORES):
        a = c * ROWS
        iter_il = np.ascontiguousarray(
            itg[:, :, a:a + TR, :]).reshape(NT, P, TR * YP)
        im2col = np.empty((NT, 36, ROWS, YP), np.float32)
        for tap in range(9):
            kx, ky = divmod(tap, 3)
            dx = kx - 1
            for c4 in range(4):
                im2col[:, tap * 4 + c4] = ing2[:, c4, a + dx + 1:a + dx + 1 + ROWS,
                                               ky:ky + YP]
        hidxa = np.zeros((P, 4), np.int32)
        pa = np.arange(P)

        def flat(rank, slot):
            return (rank * NSLOT + slot) * P + pa

        for base, dirs in ((0, 0), (2, 4)):
            hidxa[:, base] = flat(0, dirs + 1) if c == 0 else flat(c - 1, dirs + 3)
            hidxa[:, base + 1] = (flat(NC_CORES - 1, dirs + 2)
                                  if c == NC_CORES - 1 else flat(c + 1, dirs + 0))
        in_maps.append({
            "iter_il": iter_il.astype(np.float16),
            "im2col": im2col.reshape(NT, 36, ROWS * YP).astype(np.float16),
            "wh2h": wh2h.astype(np.float16), "wih": wih.astype(np.float16),
            "wi2h": wi2h.astype(np.float16), "pmat": pmat.astype(np.float16),
            "b_pre": b_pre, "b_mod": b_mod, "hidx": hidxa,
        })

    nc = _build()
    res = run_bass_kernel_spmd(nc, in_maps, core_ids=list(range(NC_CORES)),
                               trace=TRACE)
    LAST["exec_time_ns"] = res.exec_time_ns
    LAST["results"] = res

    full = np.empty((1, HID, NT, NX, NY, 2), np.float32)
    for c in range(NC_CORES):
        a = c * ROWS
        o = res.results[c]["out"]          # [NT, 128, ROWS, NY]
        full[0, :, :, a:a + ROWS, :, 0] = o[:, :HID].transpose(1, 0, 2, 3)
        full[0, :, :, a:a + ROWS, :, 1] = o[:, HID:].transpose(1, 0, 2, 3)
    return full


# revision 12
# speedup vs baseline: 1.0884x; 1.0884x over previous
"""BCRNN layer (bidirectional convolutional RNN) on 8 Trainium2 NeuronCores.

Problem: nb=1, nc=2, nt=12, nx=160, ny=160, hid=64, K=3, reflect padding,
complex conv decomposed into real convs, modReLU activation, forward +
backward temporal scans, output = sum of the two scans stacked (r, i).

Sharding: spatial rows (nx) split 8 ways (20 rows/core).  The temporal scan
runs locally per core; the 1-row conv halo of the recurrent state is
exchanged each step with an AllGather of boundary rows + per-core indirect
gather (per-core routing lives in an index input tensor so the single SPMD
program stays uniform; global-edge reflect rows are sourced the same way).

Compute strategy per conv: 9 shifted fp16 matmuls (K=128 = stacked
real/imag channels, M=128 = stacked real/imag outputs) accumulated in PSUM.
All matmul rhs operands are single CONTIGUOUS runs over the padded-row
layout (row stride == row length 162), with psum rows also 162 wide; the
two junk edge columns per row are overwritten by the reflect fixup.
i2h conv is one K=36 matmul over a host-built im2col of the 2-channel
input.  modReLU magnitude pairing (zr^2+zi^2 across partition halves) is
one extra 0/1-matrix matmul; z^2 is computed scaled by 2^-12 to stay in
fp16 range and unscaled exactly inside the Sqrt activation.
"""
import os

import numpy as np

import concourse.bass as bass
import concourse.bacc as bacc
import concourse.tile as tile
import concourse.mybir as mybir
from concourse.bass_utils import run_bass_kernel_spmd

P = 128
NC_CORES = 8
NT = 12
NX, NY = 160, 160
HID = 64
ROWS = NX // NC_CORES      # 20 owned rows per core
TR = ROWS + 2              # 22 tile rows (with halo)
YP = NY + 2                # 162 padded cols
FS = TR * YP + 2           # flat h/iter tile size (+1 guard col each side)
NSLOT = 8                  # cc slots: 4 fwd + 4 bwd boundary rows

f32 = mybir.dt.float32
f16 = mybir.dt.float16
i32 = mybir.dt.int32
AF = mybir.ActivationFunctionType
ALU = mybir.AluOpType

# conv groups (r0, nrows) over owned rows; scan order: boundary groups first
SCAN_GROUPS = [(0, 3), (17, 3), (3, 3), (6, 3), (9, 3), (12, 3), (15, 2)]
PRE_GROUPS = [(0, 3), (3, 3), (6, 3), (9, 3), (12, 3), (15, 3), (18, 2)]
CC_ROWS = [1, 2, 19, 20]   # contributed tile rows per direction
GN = 3 * YP                # max psum group width (486)

_CACHED = {}
TRACE = False
LAST = {}


def _row(q):
    """flat offset of (tile row q, padded col 0) in an FS-sized tile"""
    return 1 + q * YP


def _build():
    STAGE = int(os.environ.get("STAGE", "4"))
    if "nc" in _CACHED:
        return _CACHED["nc"]
    nc = bacc.Bacc(None, target_bir_lowering=False, debug=False,
                   num_devices=NC_CORES)

    # ---- I/O ----
    iter_il = nc.dram_tensor("iter_il", [NT, P, TR * YP], f16, kind="ExternalInput")
    im2col = nc.dram_tensor("im2col", [NT, 36, ROWS * YP], f16, kind="ExternalInput")
    wh2h = nc.dram_tensor("wh2h", [9, P, P], f16, kind="ExternalInput")
    wih = nc.dram_tensor("wih", [9, P, P], f16, kind="ExternalInput")
    wi2h = nc.dram_tensor("wi2h", [36, P], f16, kind="ExternalInput")
    pmat = nc.dram_tensor("pmat", [P, P], f16, kind="ExternalInput")
    b_pre = nc.dram_tensor("b_pre", [P, 1], f32, kind="ExternalInput")
    b_mod = nc.dram_tensor("b_mod", [P, 1], f32, kind="ExternalInput")
    hidx = nc.dram_tensor("hidx", [P, 4], i32, kind="ExternalInput")
    out = nc.dram_tensor("out", [NT, P, ROWS, NY], f32, kind="ExternalOutput")

    # ---- internal DRAM ----
    predr = nc.dram_tensor("predr", [NT, P, ROWS * YP], f32)
    sav = nc.dram_tensor("sav", [NT, P, ROWS, NY], f16)

    with tile.TileContext(nc) as tc:
        with (
            tc.tile_pool(name="wp", bufs=1) as wp,
            tc.tile_pool(name="dram", bufs=2, space="DRAM") as dram,
            tc.tile_pool(name="cps", bufs=4, space="PSUM") as cps,
            tc.tile_pool(name="mps", bufs=2, space="PSUM") as mps,
        ):
            # weights / constants
            wh = [wp.tile([P, P], f16, tag=f"wh{k}", name=f"wh{k}") for k in range(9)]
            wi = [wp.tile([P, P], f16, tag=f"wi{k}", name=f"wi{k}") for k in range(9)]
            for k in range(9):
                nc.sync.dma_start(out=wh[k][:], in_=wh2h[k])
                nc.sync.dma_start(out=wi[k][:], in_=wih[k])
            w36 = wp.tile([36, P], f16, tag="w36")
            nc.sync.dma_start(out=w36[:], in_=wi2h[:])
            pm = wp.tile([P, P], f16, tag="pm")
            nc.sync.dma_start(out=pm[:], in_=pmat[:])
            bp = wp.tile([P, 1], f32, tag="bp")
            nc.sync.dma_start(out=bp[:], in_=b_pre[:])
            bm = wp.tile([P, 1], f32, tag="bm")
            nc.sync.dma_start(out=bm[:], in_=b_mod[:])
            hix = wp.tile([P, 4], i32, tag="hix")
            nc.sync.dma_start(out=hix[:], in_=hidx[:])
            epst = wp.tile([P, 1], f32, tag="epst")
            nc.vector.memset(epst[:], 1e-6)

            # ---------------- phase 1: pre[t] = ih(iter) + i2h(input) + bias
            with (
                tc.tile_pool(name="itp", bufs=2) as itp,
                tc.tile_pool(name="icp", bufs=2) as icp,
                tc.tile_pool(name="prs", bufs=2) as prs,
            ):
                for t in range(NT):
                    it = itp.tile([P, FS], f16, tag="it")
                    nc.sync.dma_start(out=it[:, 1:1 + TR * YP], in_=iter_il[t])
                    ic = icp.tile([36, ROWS * YP], f16, tag="ic")
                    nc.sync.dma_start(out=ic[:], in_=im2col[t])
                    stage = prs.tile([P, ROWS * YP], f32, tag="stage")
                    for r0, nr in PRE_GROUPS:
                        n = nr * YP
                        ps = cps.tile([P, GN], f32, tag="cv")
                        for tap in range(9):
                            dxi, dyi = divmod(tap, 3)
                            base = (r0 + dxi) * YP + dyi
                            nc.tensor.matmul(
                                out=ps[:, :n],
                                lhsT=wi[tap][:],
                                rhs=it[:, base:base + n],
                                start=(tap == 0), stop=False,
                            )
                        nc.tensor.matmul(
                            out=ps[:, :n],
                            lhsT=w36[:],
                            rhs=ic[:, r0 * YP:r0 * YP + n],
                            start=False, stop=True,
                        )
                        nc.scalar.activation(stage[:, r0 * YP:r0 * YP + n],
                                             ps[:, :n],
                                             AF.Identity, bias=bp[:, 0:1], scale=1.0)
                    nc.sync.dma_start(out=predr[t], in_=stage[:])

            # ---------------- phase 2: bidirectional scan
            with (
                tc.tile_pool(name="hfp", bufs=2) as hfp,
                tc.tile_pool(name="hbp", bufs=2) as hbp,
                tc.tile_pool(name="pin", bufs=3) as pinp,
                tc.tile_pool(name="zp", bufs=2) as zp,
                tc.tile_pool(name="z2p", bufs=2) as z2p,
                tc.tile_pool(name="mgp", bufs=2) as mgp,
                tc.tile_pool(name="nmp", bufs=2) as nmp,
                tc.tile_pool(name="svp", bufs=2) as svp,
                tc.tile_pool(name="obp", bufs=2) as obp,
            ):
                hf_prev = hb_prev = None
                for s in range(NT if STAGE > 1 else 0):
                    new_state = {}
                    for dire, (hpool, h_prev) in (("f", (hfp, hf_prev)),
                                                  ("b", (hbp, hb_prev))):
                        t_d = s if dire == "f" else NT - 1 - s
                        pin = pinp.tile([P, ROWS * YP], f32, tag="pin")
                        nc.sync.dma_start(out=pin[:], in_=predr[t_d])
                        h_next = hpool.tile([P, FS], f16, tag=f"h{dire}")
                        if STAGE == 2:
                            nc.vector.memset(h_next[:, _row(0):_row(1)], 0.0)
                            nc.vector.memset(h_next[:, _row(TR - 1):_row(TR)], 0.0)
                        for r0, nr in SCAN_GROUPS:
                            n = nr * YP
                            if s > 0:
                                ps = cps.tile([P, GN], f32, tag="cv")
                                for tap in range(9):
                                    dxi, dyi = divmod(tap, 3)
                                    base = (r0 + dxi) * YP + dyi
                                    nc.tensor.matmul(
                                        out=ps[:, :n],
                                        lhsT=wh[tap][:],
                                        rhs=h_prev[:, base:base + n],
                                        start=(tap == 0), stop=(tap == 8),
                                    )
                                z = zp.tile([P, GN], f32, tag="z")
                                nc.vector.tensor_tensor(
                                    out=z[:, :n], in0=ps[:, :n],
                                    in1=pin[:, r0 * YP:r0 * YP + n], op=ALU.add)
                                zv = z[:, :n]
                            else:
                                zv = pin[:, r0 * YP:r0 * YP + n]
                            z2 = z2p.tile([P, GN], f16, tag="z2")
                            nc.scalar.activation(z2[:, :n], zv, AF.Square,
                                                 scale=0.015625)
                            mp = mps.tile([P, GN], f32, tag="mg")
                            nc.tensor.matmul(out=mp[:, :n], lhsT=pm[:],
                                             rhs=z2[:, :n],
                                             start=True, stop=True)
                            # rs = 1/mag; scale = max(0, 1 + b/mag);
                            # h = scale * z  (equivalent to relu(mag+b)/mag)
                            rs = mgp.tile([P, GN], f32, tag="rs")
                            nc.scalar.activation(rs[:, :n], mp[:, :n],
                                                 AF.Abs_reciprocal_sqrt,
                                                 bias=epst[:, 0:1],
                                                 scale=4096.0)
                            q = nmp.tile([P, GN], f32, tag="q")
                            nc.vector.tensor_scalar(
                                out=q[:, :n], in0=rs[:, :n],
                                scalar1=bm[:, 0:1], scalar2=1.0,
                                op0=ALU.mult, op1=ALU.add)
                            hb_ = _row(r0 + 1)
                            nc.vector.scalar_tensor_tensor(
                                out=h_next[:, hb_:hb_ + n],
                                in0=q[:, :n], scalar=0.0, in1=zv,
                                op0=ALU.max, op1=ALU.mult)
                        if s < NT - 1:
                            # y-edge reflect on owned rows (also fixes junk
                            # psum edge columns)
                            vr = h_next[:, _row(1):_row(TR - 1)].rearrange(
                                "p (r y) -> p r y", y=YP)
                            nc.vector.tensor_copy(out=vr[:, :, 0:1],
                                                  in_=vr[:, :, 2:3])
                            nc.vector.tensor_copy(out=vr[:, :, YP - 1:YP],
                                                  in_=vr[:, :, YP - 3:YP - 2])
                        new_state[dire] = h_next

                    hf_next, hb_next = new_state["f"], new_state["b"]

                    if s < NT - 1 and STAGE >= 3:
                        # halo exchange
                        cc_in = dram.tile([NSLOT * P, YP], f16, tag="cci")
                        cc_out = dram.tile([NC_CORES * NSLOT * P, YP], f16,
                                           addr_space="Shared", tag="cco")
                        cci_v = cc_in[:].rearrange("(s p) y -> s p y", p=P)
                        for di, h_n in ((0, hf_next), (1, hb_next)):
                            for si, row in enumerate(CC_ROWS):
                                nc.sync.dma_start(
                                    out=cci_v[di * 4 + si],
                                    in_=h_n[:, _row(row):_row(row + 1)])
                        nc.gpsimd.collective_compute(
                            "AllGather", ALU.bypass,
                            replica_groups=[list(range(NC_CORES))],
                            ins=[cc_in[:].opt()], outs=[cc_out[:].opt()],
                        )
                        for k, (h_n, row) in enumerate(
                                ((hf_next, 0), (hf_next, TR - 1),
                                 (hb_next, 0), (hb_next, TR - 1))):
                            nc.gpsimd.indirect_dma_start(
                                out=h_n[:, _row(row):_row(row + 1)],
                                out_offset=None,
                                in_=cc_out[:],
                                in_offset=bass.IndirectOffsetOnAxis(
                                    ap=hix[:, k:k + 1], axis=0),
                            )

                    # save / combine
                    def _ivw(h_n):
                        return h_n[:, _row(1):_row(TR - 1)].rearrange(
                            "p (r y) -> p r y", y=YP)[:, :, 1:1 + NY]

                    if s <= 5:
                        nc.sync.dma_start(out=sav[s], in_=_ivw(hf_next))
                        nc.sync.dma_start(out=sav[NT - 1 - s], in_=_ivw(hb_next))
                    else:
                        for h_n, t_o in ((hf_next, s), (hb_next, NT - 1 - s)):
                            ld = svp.tile([P, ROWS, NY], f16, tag="ld")
                            nc.sync.dma_start(out=ld[:], in_=sav[t_o])
                            ob = obp.tile([P, ROWS, NY], f32, tag="ob")
                            nc.vector.tensor_tensor(
                                out=ob[:], in0=_ivw(h_n),
                                in1=ld[:], op=ALU.add)
                            nc.sync.dma_start(out=out[t_o], in_=ob[:])

                    hf_prev, hb_prev = hf_next, hb_next

    nc.compile()
    _CACHED["nc"] = nc
    return nc


def _complex_lhsT(wr, wi_):
    """[O, I, 3, 3] complex pair -> per-tap lhsT [9, 2*I, 2*O]."""
    O, I = wr.shape[:2]
    lhsT = np.zeros((9, 2 * I, 2 * O), np.float32)
    for tap in range(9):
        kx, ky = divmod(tap, 3)
        lhsT[tap, :I, :O] = wr[:, :, kx, ky].T
        lhsT[tap, I:, :O] = -wi_[:, :, kx, ky].T
        lhsT[tap, :I, O:] = wi_[:, :, kx, ky].T
        lhsT[tap, I:, O:] = wr[:, :, kx, ky].T
    return lhsT


def kernel(**inputs):
    inp_r = np.asarray(inputs["input_r"], np.float32)
    inp_i = np.asarray(inputs["input_i"], np.float32)
    itr_r = np.asarray(inputs["iter_r"], np.float32)
    itr_i = np.asarray(inputs["iter_i"], np.float32)

    # ---- weights ----
    wh2h = _complex_lhsT(np.asarray(inputs["w_h2h_r"]), np.asarray(inputs["w_h2h_i"]))
    wih = _complex_lhsT(np.asarray(inputs["w_ih_r"]), np.asarray(inputs["w_ih_i"]))
    w4 = _complex_lhsT(np.asarray(inputs["w_i2h_r"]), np.asarray(inputs["w_i2h_i"]))
    wi2h = np.ascontiguousarray(w4.reshape(36, P))
    pmat = np.zeros((P, P), np.float32)
    for k in range(P):
        pmat[k, k % HID] = 1.0
        pmat[k, HID + k % HID] = 1.0
    b_pre = np.concatenate([
        inputs["b_i2h_r"] + inputs["b_ih_r"] + inputs["b_h2h_r"],
        inputs["b_i2h_i"] + inputs["b_ih_i"] + inputs["b_h2h_i"],
    ]).astype(np.float32)[:, None]
    b_mod = np.tile(np.asarray(inputs["mod_b"], np.float32), 2)[:, None]

    # ---- activations, reflect-padded [t, ch, xpad, ypad] ----
    itg = np.concatenate([itr_r[0], itr_i[0]], axis=0).transpose(1, 0, 2, 3)
    itg = np.pad(itg, ((0, 0), (0, 0), (1, 1), (1, 1)), mode="reflect")
    ing = np.concatenate([inp_r[0], inp_i[0]], axis=0).transpose(1, 0, 2, 3)
    ing = np.pad(ing, ((0, 0), (0, 0), (1, 1), (1, 1)), mode="reflect")
    # one extra (junk) col each side for the 162-wide im2col windows
    ing2 = np.pad(ing, ((0, 0), (0, 0), (0, 0), (1, 1)), mode="edge")

    in_maps = []
    for c in range(NC_CORES):
        a = c * ROWS
        iter_il = np.ascontiguousarray(
            itg[:, :, a:a + TR, :]).reshape(NT, P, TR * YP)
        im2col = np.empty((NT, 36, ROWS, YP), np.float32)
        for tap in range(9):
            kx, ky = divmod(tap, 3)
            dx = kx - 1
            for c4 in range(4):
                im2col[:, tap * 4 + c4] = ing2[:, c4, a + dx + 1:a + dx + 1 + ROWS,
                                               ky:ky + YP]
        hidxa = np.zeros((P, 4), np.int32)
        pa = np.arange(P)

        def flat(rank, slot):
            return (rank * NSLOT + slot) * P + pa

        for base, dirs in ((0, 0), (2, 4)):
            hidxa[:, base] = flat(0, dirs + 1) if c == 0 else flat(c - 1, dirs + 3)
            hidxa[:, base + 1] = (flat(NC_CORES - 1, dirs + 2)
                                  if c == NC_CORES - 1 else flat(c + 1, dirs + 0))
        in_maps.append({
            "iter_il": iter_il.astype(np.float16),
            "im2col": im2col.reshape(NT, 36, ROWS * YP).astype(np.float16),
            "wh2h": wh2h.astype(np.float16), "wih": wih.astype(np.float16),
            "wi2h": wi2h.astype(np.float16), "pmat": pmat.astype(np.float16),
            "b_pre": b_pre, "b_mod": b_mod, "hidx": hidxa,
        })

    nc = _build()
    res = run_bass_kernel_spmd(nc, in_maps, core_ids=list(range(NC_CORES)),
                               trace=TRACE)
    LAST["exec_time_ns"] = res.exec_time_ns
    LAST["results"] = res

    full = np.empty((1, HID, NT, NX, NY, 2), np.float32)
    for c in range(NC_CORES):
        a = c * ROWS
        o = res.results[c]["out"]          # [NT, 128, ROWS, NY]
        full[0, :, :, a:a + ROWS, :, 0] = o[:, :HID].transpose(1, 0, 2, 3)
        full[0, :, :, a:a + ROWS, :, 1] = o[:, HID:].transpose(1, 0, 2, 3)
    return full


# revision 13
# speedup vs baseline: 1.2106x; 1.1123x over previous
"""BCRNN layer (bidirectional convolutional RNN) on 8 Trainium2 NeuronCores.

Problem: nb=1, nc=2, nt=12, nx=160, ny=160, hid=64, K=3, reflect padding,
complex conv decomposed into real convs, modReLU activation, forward +
backward temporal scans, output = sum of the two scans stacked (r, i).

Sharding: spatial rows (nx) split 8 ways (20 rows/core).  The temporal scan
runs locally per core; the 1-row conv halo of the recurrent state is
exchanged each step with an AllGather of boundary rows + per-core indirect
gather (per-core routing lives in an index input tensor so the single SPMD
program stays uniform; global-edge reflect rows are sourced the same way).

Compute strategy per conv: 9 shifted fp16 matmuls (K=128 = stacked
real/imag channels, M=128 = stacked real/imag outputs) accumulated in PSUM.
All matmul rhs operands are single CONTIGUOUS runs over the padded-row
layout (row stride == row length 162), with psum rows also 162 wide; the
two junk edge columns per row are overwritten by the reflect fixup.
i2h conv is one K=36 matmul over a host-built im2col of the 2-channel
input.  modReLU magnitude pairing (zr^2+zi^2 across partition halves) is
one extra 0/1-matrix matmul; z^2 is computed scaled by 2^-12 to stay in
fp16 range and unscaled exactly inside the Sqrt activation.
"""
import os

import numpy as np

import concourse.bass as bass
import concourse.bacc as bacc
import concourse.tile as tile
import concourse.mybir as mybir
from concourse.bass_utils import run_bass_kernel_spmd

P = 128
NC_CORES = 8
NT = 12
NX, NY = 160, 160
HID = 64
ROWS = NX // NC_CORES      # 20 owned rows per core
TR = ROWS + 2              # 22 tile rows (with halo)
YP = NY + 2                # 162 padded cols
FS = TR * YP + 2           # flat h/iter tile size (+1 guard col each side)
NSLOT = 8                  # cc slots: 4 fwd + 4 bwd boundary rows

f32 = mybir.dt.float32
f16 = mybir.dt.float16
i32 = mybir.dt.int32
AF = mybir.ActivationFunctionType
ALU = mybir.AluOpType

# conv groups (r0, nrows) over owned rows; scan order: boundary groups first
SCAN_GROUPS = [(0, 3), (17, 3), (3, 3), (6, 3), (9, 3), (12, 3), (15, 2)]
PRE_GROUPS = [(0, 3), (3, 3), (6, 3), (9, 3), (12, 3), (15, 3), (18, 2)]
CC_ROWS = [1, 2, 19, 20]   # contributed tile rows per direction
GN = 3 * YP                # max psum group width (486)

_CACHED = {}
TRACE = False
LAST = {}


def _row(q):
    """flat offset of (tile row q, padded col 0) in an FS-sized tile"""
    return 1 + q * YP


def _build():
    STAGE = int(os.environ.get("STAGE", "4"))
    if "nc" in _CACHED:
        return _CACHED["nc"]
    nc = bacc.Bacc(None, target_bir_lowering=False, debug=False,
                   num_devices=NC_CORES)

    # ---- I/O ----
    iter_il = nc.dram_tensor("iter_il", [NT, P, TR * YP], f16, kind="ExternalInput")
    im2col = nc.dram_tensor("im2col", [NT, 36, ROWS * YP], f16, kind="ExternalInput")
    wh2h = nc.dram_tensor("wh2h", [9, P, P], f16, kind="ExternalInput")
    wih = nc.dram_tensor("wih", [9, P, P], f16, kind="ExternalInput")
    wi2h = nc.dram_tensor("wi2h", [36, P], f16, kind="ExternalInput")
    pmat = nc.dram_tensor("pmat", [P, P], f16, kind="ExternalInput")
    imat = nc.dram_tensor("imat", [P, P], f16, kind="ExternalInput")
    b_pre = nc.dram_tensor("b_pre", [P, 1], f32, kind="ExternalInput")
    b_mod = nc.dram_tensor("b_mod", [P, 1], f32, kind="ExternalInput")
    hidx = nc.dram_tensor("hidx", [P, 4], i32, kind="ExternalInput")
    out = nc.dram_tensor("out", [NT, P, ROWS, NY], f32, kind="ExternalOutput")

    # ---- internal DRAM ----
    predr = nc.dram_tensor("predr", [NT, P, ROWS * YP], f16)
    sav = nc.dram_tensor("sav", [NT, P, ROWS, NY], f16)

    with tile.TileContext(nc) as tc:
        with (
            tc.tile_pool(name="wp", bufs=1) as wp,
            tc.tile_pool(name="dram", bufs=2, space="DRAM") as dram,
            tc.tile_pool(name="cps", bufs=6, space="PSUM") as cps,
            tc.tile_pool(name="mps", bufs=2, space="PSUM") as mps,
        ):
            # weights / constants
            wh = [wp.tile([P, P], f16, tag=f"wh{k}", name=f"wh{k}") for k in range(9)]
            wi = [wp.tile([P, P], f16, tag=f"wi{k}", name=f"wi{k}") for k in range(9)]
            for k in range(9):
                nc.sync.dma_start(out=wh[k][:], in_=wh2h[k])
                nc.sync.dma_start(out=wi[k][:], in_=wih[k])
            w36 = wp.tile([36, P], f16, tag="w36")
            nc.sync.dma_start(out=w36[:], in_=wi2h[:])
            pm = wp.tile([P, P], f16, tag="pm")
            nc.sync.dma_start(out=pm[:], in_=pmat[:])
            im = wp.tile([P, P], f16, tag="im")
            nc.sync.dma_start(out=im[:], in_=imat[:])
            bp = wp.tile([P, 1], f32, tag="bp")
            nc.sync.dma_start(out=bp[:], in_=b_pre[:])
            bm = wp.tile([P, 1], f32, tag="bm")
            nc.sync.dma_start(out=bm[:], in_=b_mod[:])
            hix = wp.tile([P, 4], i32, tag="hix")
            nc.sync.dma_start(out=hix[:], in_=hidx[:])
            epst = wp.tile([P, 1], f32, tag="epst")
            nc.vector.memset(epst[:], 1e-6)

            # ---------------- phase 1: pre[t] = ih(iter) + i2h(input) + bias
            with (
                tc.tile_pool(name="itp", bufs=2) as itp,
                tc.tile_pool(name="icp", bufs=2) as icp,
                tc.tile_pool(name="prs", bufs=2) as prs,
            ):
                for t in range(NT):
                    it = itp.tile([P, FS], f16, tag="it")
                    nc.sync.dma_start(out=it[:, 1:1 + TR * YP], in_=iter_il[t])
                    ic = icp.tile([36, ROWS * YP], f16, tag="ic")
                    nc.sync.dma_start(out=ic[:], in_=im2col[t])
                    stage = prs.tile([P, ROWS * YP], f16, tag="stage")
                    for r0, nr in PRE_GROUPS:
                        n = nr * YP
                        ps = cps.tile([P, GN], f32, tag="cv")
                        for tap in range(9):
                            dxi, dyi = divmod(tap, 3)
                            base = (r0 + dxi) * YP + dyi
                            nc.tensor.matmul(
                                out=ps[:, :n],
                                lhsT=wi[tap][:],
                                rhs=it[:, base:base + n],
                                start=(tap == 0), stop=False,
                            )
                        nc.tensor.matmul(
                            out=ps[:, :n],
                            lhsT=w36[:],
                            rhs=ic[:, r0 * YP:r0 * YP + n],
                            start=False, stop=True,
                        )
                        nc.scalar.activation(stage[:, r0 * YP:r0 * YP + n],
                                             ps[:, :n],
                                             AF.Identity, bias=bp[:, 0:1], scale=1.0)
                    nc.sync.dma_start(out=predr[t], in_=stage[:])

            # ---------------- phase 2: bidirectional scan
            with (
                tc.tile_pool(name="hfp", bufs=2) as hfp,
                tc.tile_pool(name="hbp", bufs=2) as hbp,
                tc.tile_pool(name="pin", bufs=4) as pinp,
                tc.tile_pool(name="z2p", bufs=2) as z2p,
                tc.tile_pool(name="mgp", bufs=2) as mgp,
                tc.tile_pool(name="nmp", bufs=2) as nmp,
                tc.tile_pool(name="svp", bufs=2) as svp,
                tc.tile_pool(name="obp", bufs=2) as obp,
            ):
                hf_prev = hb_prev = None
                for s in range(NT if STAGE > 1 else 0):
                    new_state = {}
                    for dire, (hpool, h_prev) in (("f", (hfp, hf_prev)),
                                                  ("b", (hbp, hb_prev))):
                        t_d = s if dire == "f" else NT - 1 - s
                        pin = pinp.tile([P, ROWS * YP], f16, tag="pin")
                        nc.sync.dma_start(out=pin[:], in_=predr[t_d])
                        h_next = hpool.tile([P, FS], f16, tag=f"h{dire}")
                        if STAGE == 2:
                            nc.vector.memset(h_next[:, _row(0):_row(1)], 0.0)
                            nc.vector.memset(h_next[:, _row(TR - 1):_row(TR)], 0.0)
                        for r0, nr in SCAN_GROUPS:
                            n = nr * YP
                            ps = cps.tile([P, GN], f32, tag="cv")
                            if s > 0:
                                for tap in range(9):
                                    dxi, dyi = divmod(tap, 3)
                                    base = (r0 + dxi) * YP + dyi
                                    nc.tensor.matmul(
                                        out=ps[:, :n],
                                        lhsT=wh[tap][:],
                                        rhs=h_prev[:, base:base + n],
                                        start=(tap == 0), stop=False,
                                    )
                            nc.tensor.matmul(
                                out=ps[:, :n], lhsT=im[:],
                                rhs=pin[:, r0 * YP:r0 * YP + n],
                                start=(s == 0), stop=True,
                            )
                            zv = ps[:, :n]
                            z2 = z2p.tile([P, GN], f16, tag="z2")
                            nc.scalar.activation(z2[:, :n], zv, AF.Square,
                                                 scale=0.015625)
                            mp = mps.tile([P, GN], f32, tag="mg")
                            nc.tensor.matmul(out=mp[:, :n], lhsT=pm[:],
                                             rhs=z2[:, :n],
                                             start=True, stop=True)
                            # rs = 1/mag; scale = max(0, 1 + b/mag);
                            # h = scale * z  (equivalent to relu(mag+b)/mag)
                            rs = mgp.tile([P, GN], f32, tag="rs")
                            nc.scalar.activation(rs[:, :n], mp[:, :n],
                                                 AF.Abs_reciprocal_sqrt,
                                                 bias=epst[:, 0:1],
                                                 scale=4096.0)
                            q = nmp.tile([P, GN], f32, tag="q")
                            nc.vector.tensor_scalar(
                                out=q[:, :n], in0=rs[:, :n],
                                scalar1=bm[:, 0:1], scalar2=1.0,
                                op0=ALU.mult, op1=ALU.add)
                            hb_ = _row(r0 + 1)
                            nc.vector.scalar_tensor_tensor(
                                out=h_next[:, hb_:hb_ + n],
                                in0=q[:, :n], scalar=0.0, in1=zv,
                                op0=ALU.max, op1=ALU.mult)
                            if s < NT - 1:
                                # per-group y-edge reflect (also overwrites
                                # junk psum edge columns)
                                vr = h_next[:, hb_:hb_ + n].rearrange(
                                    "p (r y) -> p r y", y=YP)
                                nc.vector.tensor_copy(out=vr[:, :, 0:1],
                                                      in_=vr[:, :, 2:3])
                                nc.vector.tensor_copy(
                                    out=vr[:, :, YP - 1:YP],
                                    in_=vr[:, :, YP - 3:YP - 2])
                        new_state[dire] = h_next

                    hf_next, hb_next = new_state["f"], new_state["b"]

                    if s < NT - 1 and STAGE >= 3:
                        # halo exchange
                        cc_in = dram.tile([NSLOT * P, YP], f16, tag="cci")
                        cc_out = dram.tile([NC_CORES * NSLOT * P, YP], f16,
                                           addr_space="Shared", tag="cco")
                        cci_v = cc_in[:].rearrange("(s p) y -> s p y", p=P)
                        for di, h_n in ((0, hf_next), (1, hb_next)):
                            for si, row in enumerate(CC_ROWS):
                                nc.sync.dma_start(
                                    out=cci_v[di * 4 + si],
                                    in_=h_n[:, _row(row):_row(row + 1)])
                        nc.gpsimd.collective_compute(
                            "AllGather", ALU.bypass,
                            replica_groups=[list(range(NC_CORES))],
                            ins=[cc_in[:].opt()], outs=[cc_out[:].opt()],
                        )
                        for k, (h_n, row) in enumerate(
                                ((hf_next, 0), (hf_next, TR - 1),
                                 (hb_next, 0), (hb_next, TR - 1))):
                            nc.gpsimd.indirect_dma_start(
                                out=h_n[:, _row(row):_row(row + 1)],
                                out_offset=None,
                                in_=cc_out[:],
                                in_offset=bass.IndirectOffsetOnAxis(
                                    ap=hix[:, k:k + 1], axis=0),
                            )

                    # save / combine
                    def _ivw(h_n):
                        return h_n[:, _row(1):_row(TR - 1)].rearrange(
                            "p (r y) -> p r y", y=YP)[:, :, 1:1 + NY]

                    if s <= 5:
                        nc.sync.dma_start(out=sav[s], in_=_ivw(hf_next))
                        nc.sync.dma_start(out=sav[NT - 1 - s], in_=_ivw(hb_next))
                    else:
                        for h_n, t_o in ((hf_next, s), (hb_next, NT - 1 - s)):
                            ld = svp.tile([P, ROWS, NY], f16, tag="ld")
                            nc.sync.dma_start(out=ld[:], in_=sav[t_o])
                            ob = obp.tile([P, ROWS, NY], f32, tag="ob")
                            nc.vector.tensor_tensor(
                                out=ob[:], in0=_ivw(h_n),
                                in1=ld[:], op=ALU.add)
                            nc.sync.dma_start(out=out[t_o], in_=ob[:])

                    hf_prev, hb_prev = hf_next, hb_next

    nc.compile()
    _CACHED["nc"] = nc
    return nc


def _complex_lhsT(wr, wi_):
    """[O, I, 3, 3] complex pair -> per-tap lhsT [9, 2*I, 2*O]."""
    O, I = wr.shape[:2]
    lhsT = np.zeros((9, 2 * I, 2 * O), np.float32)
    for tap in range(9):
        kx, ky = divmod(tap, 3)
        lhsT[tap, :I, :O] = wr[:, :, kx, ky].T
        lhsT[tap, I:, :O] = -wi_[:, :, kx, ky].T
        lhsT[tap, :I, O:] = wi_[:, :, kx, ky].T
        lhsT[tap, I:, O:] = wr[:, :, kx, ky].T
    return lhsT


def kernel(**inputs):
    inp_r = np.asarray(inputs["input_r"], np.float32)
    inp_i = np.asarray(inputs["input_i"], np.float32)
    itr_r = np.asarray(inputs["iter_r"], np.float32)
    itr_i = np.asarray(inputs["iter_i"], np.float32)

    # ---- weights ----
    wh2h = _complex_lhsT(np.asarray(inputs["w_h2h_r"]), np.asarray(inputs["w_h2h_i"]))
    wih = _complex_lhsT(np.asarray(inputs["w_ih_r"]), np.asarray(inputs["w_ih_i"]))
    w4 = _complex_lhsT(np.asarray(inputs["w_i2h_r"]), np.asarray(inputs["w_i2h_i"]))
    wi2h = np.ascontiguousarray(w4.reshape(36, P))
    pmat = np.zeros((P, P), np.float32)
    for k in range(P):
        pmat[k, k % HID] = 1.0
        pmat[k, HID + k % HID] = 1.0
    b_pre = np.concatenate([
        inputs["b_i2h_r"] + inputs["b_ih_r"] + inputs["b_h2h_r"],
        inputs["b_i2h_i"] + inputs["b_ih_i"] + inputs["b_h2h_i"],
    ]).astype(np.float32)[:, None]
    b_mod = np.tile(np.asarray(inputs["mod_b"], np.float32), 2)[:, None]

    # ---- activations, reflect-padded [t, ch, xpad, ypad] ----
    itg = np.concatenate([itr_r[0], itr_i[0]], axis=0).transpose(1, 0, 2, 3)
    itg = np.pad(itg, ((0, 0), (0, 0), (1, 1), (1, 1)), mode="reflect")
    ing = np.concatenate([inp_r[0], inp_i[0]], axis=0).transpose(1, 0, 2, 3)
    ing = np.pad(ing, ((0, 0), (0, 0), (1, 1), (1, 1)), mode="reflect")
    # one extra (junk) col each side for the 162-wide im2col windows
    ing2 = np.pad(ing, ((0, 0), (0, 0), (0, 0), (1, 1)), mode="edge")

    in_maps = []
    for c in range(NC_CORES):
        a = c * ROWS
        iter_il = np.ascontiguousarray(
            itg[:, :, a:a + TR, :]).reshape(NT, P, TR * YP)
        im2col = np.empty((NT, 36, ROWS, YP), np.float32)
        for tap in range(9):
            kx, ky = divmod(tap, 3)
            dx = kx - 1
            for c4 in range(4):
                im2col[:, tap * 4 + c4] = ing2[:, c4, a + dx + 1:a + dx + 1 + ROWS,
                                               ky:ky + YP]
        hidxa = np.zeros((P, 4), np.int32)
        pa = np.arange(P)

        def flat(rank, slot):
            return (rank * NSLOT + slot) * P + pa

        for base, dirs in ((0, 0), (2, 4)):
            hidxa[:, base] = flat(0, dirs + 1) if c == 0 else flat(c - 1, dirs + 3)
            hidxa[:, base + 1] = (flat(NC_CORES - 1, dirs + 2)
                                  if c == NC_CORES - 1 else flat(c + 1, dirs + 0))
        in_maps.append({
            "imat": np.eye(P, dtype=np.float16),
            "iter_il": iter_il.astype(np.float16),
            "im2col": im2col.reshape(NT, 36, ROWS * YP).astype(np.float16),
            "wh2h": wh2h.astype(np.float16), "wih": wih.astype(np.float16),
            "wi2h": wi2h.astype(np.float16), "pmat": pmat.astype(np.float16),
            "b_pre": b_pre, "b_mod": b_mod, "hidx": hidxa,
        })

    nc = _build()
    res = run_bass_kernel_spmd(nc, in_maps, core_ids=list(range(NC_CORES)),
                               trace=TRACE)
    LAST["exec_time_ns"] = res.exec_time_ns
    LAST["results"] = res

    full = np.empty((1, HID, NT, NX, NY, 2), np.float32)
    for c in range(NC_CORES):
        a = c * ROWS
        o = res.results[c]["out"]          # [NT, 128, ROWS, NY]
        full[0, :, :, a:a + ROWS, :, 0] = o[:, :HID].transpose(1, 0, 2, 3)
        full[0, :, :, a:a + ROWS, :, 1] = o[:, HID:].transpose(1, 0, 2, 3)
    return full
